# revision 7
# baseline (speedup 1.0000x reference)
"""Grouped channel self-interaction kernel for Trainium2 (8 NeuronCores).

out[b, c] = inp[b, c] * (sum of inp[b, c'] over c' in c's group of 8) / 32

Input [32, 256, 56, 56] f32. Sharding: data-parallel over batch, 4 batches
per core. Per core the slice is viewed as [128, 8, 3136]: partition rows are
(batch, group) pairs (4*32 = 128 exactly), free axis is (channel-in-group,
spatial). Every partition row is fully contiguous in DRAM.

Uneven spatial chunks [392, 784, 784, 784, 392]: the half-size first chunk
lets VectorE start ~4us earlier (shorter first in-DMA), and the half-size
last chunk shrinks the end-of-kernel compute+store tail. Per chunk, 7 DVE
adds build the group sum in PSUM, then ONE scalar_tensor_tensor with the
accumulator broadcast (stride-0) across the 8-channel axis computes
(x * 1/32) * group_sum for the whole tile. Two pools per memory space so
slot sizes match tile sizes (one pool would size every slot at the 784 max
and overflow the 208KB/partition SBUF budget). Measured via neuron-profile
(NTFF): ~80us per execution, against a ~62us pure-DMA floor at the
observed 413 GB/s per-core HBM rate plus ~10us NRT pre/postamble.
"""

import numpy as np

_B, _C, _H, _W = 32, 256, 56, 56
_S = _H * _W              # 3136
_NCORES = 8
_BPC = _B // _NCORES      # 4 batches per core
_G = 32                   # groups
_CPG = 8                  # channels per group
_SCALE = 1.0 / 32.0       # 1 / NUM_GROUPS

_CHUNKS = [392, 784, 784, 784, 392]

_cache: dict = {}


def _build_nc(reps: int = 1):
    import concourse.bacc as bacc
    import concourse.mybir as mybir
    from concourse.tile import TileContext

    f32 = mybir.dt.float32
    mult = mybir.AluOpType.mult
    # Bacc (not raw Bass): its compile() runs generate_event_semaphores(),
    # which splits sync waits to satisfy the 1-wait-per-instruction HW limit.
    nc = bacc.Bacc()
    x = nc.dram_tensor("inp", [128, _CPG, _S], f32, kind="ExternalInput")
    y = nc.dram_tensor("out", [128, _CPG, _S], f32, kind="ExternalOutput")

    with TileContext(nc) as tc:
        with (
            tc.tile_pool(name="xb", bufs=3) as xb,
            tc.tile_pool(name="xs", bufs=2) as xs,
            # acc lives in PSUM (otherwise unused; DVE reads PSUM at full
            # f32 rate). A 784-col f32 acc takes 2 of the 8 banks, a 392-col
            # acc takes 1: 3*2 + 2*1 = 8 banks exactly.
            tc.tile_pool(name="ab", bufs=3, space="PSUM") as ab,
            tc.tile_pool(name="as_", bufs=2, space="PSUM") as asml,
            tc.tile_pool(name="yb", bufs=3) as yb,
            tc.tile_pool(name="ys", bufs=2) as ys,
        ):
            # reps>1 is a timing-only variant: the identical full-input body
            # repeated back to back (each rep re-reads inp from DRAM and
            # re-writes out), so marginal time per rep = device time of one
            # full execution with host/dispatch overhead excluded.
            for _ in range(reps):
                off = 0
                for ch in _CHUNKS:
                    sl = slice(off, off + ch)
                    off += ch
                    big = ch == 784
                    xt = (xb if big else xs).tile([128, _CPG, ch], f32)
                    nc.sync.dma_start(xt[:], x[:, :, sl])
                    acc = (ab if big else asml).tile([128, ch], f32)
                    nc.vector.tensor_add(acc[:], xt[:, 0, :], xt[:, 1, :])
                    for c in range(2, _CPG):
                        nc.vector.tensor_add(acc[:], acc[:], xt[:, c, :])
                    ot = (yb if big else ys).tile([128, _CPG, ch], f32)
                    accb = acc[:].unsqueeze(1).broadcast_to([128, _CPG, ch])
                    nc.vector.scalar_tensor_tensor(
                        ot[:], xt[:], _SCALE, accb, mult, mult
                    )
                    nc.sync.dma_start(y[:, :, sl], ot[:])
    nc.compile()
    return nc


def _in_maps(inp: np.ndarray) -> list:
    x = np.ascontiguousarray(inp, dtype=np.float32).reshape(
        _NCORES, _BPC * _G, _CPG, _S
    )
    return [{"inp": x[i]} for i in range(_NCORES)]


def kernel(inp: np.ndarray) -> np.ndarray:
    from concourse.bass_utils import run_bass_kernel_spmd

    if "nc" not in _cache:
        _cache["nc"] = _build_nc()
    res = run_bass_kernel_spmd(_cache["nc"], _in_maps(inp), list(range(_NCORES)))
    out = np.stack([np.asarray(res.results[i]["out"]) for i in range(_NCORES)])
    return out.reshape(_B, _C, _H, _W)
